# revision 5
# baseline (speedup 1.0000x reference)
"""Causal depthwise conv1d (B=4, T=8192, C=2048, K=4) on 8 Trainium2 cores.

Sharding: 8 shards = (batch b, T-half h), each core computes out[b, h*4096:(h+1)*4096, :].
Halo handled host-side: each core's input is 4224 rows of a zero-padded copy of x,
so row i of the shard is x[b, t0 + i - 3] (zeros outside [0, T)).

Per-core kernel (all fp16 on-chip, fp32 in HBM):
  - SWDGE DMA loads [t,c] chunks with fp32->fp16 cast (contiguous 2KB HBM reads)
  - PE transposes 128x128 chunks into PSUM => xT[c_part, t_free]
  - MAC with per-partition scalars: out[c,t] = sum_k w_k[c]*xT[c,t+k] + bias[c]
      ACT takes odd offsets (alignment-immune), DVE takes even offsets via
      fused scalar_tensor_tensor; all tensor operands 16-bit => 2x/4x DVE modes
  - PE transposes the result back to [t,c], DVE evacuates PSUM
  - SWDGE DMA stores with fp16->fp32 cast
"""

import sys

if "/opt/trn_rl_repo" not in sys.path:
    sys.path.insert(0, "/opt/trn_rl_repo")

import numpy as np

B, T, C, K = 4, 8192, 2048, 4
N_CORES = 8
TL = T // 2            # 4096 rows of output per core
HALO = K - 1           # 3
PAD_ROWS = TL + 128    # 4224 input rows per core (halo + data + tail pad)
T_HALF = 2048          # time rows per pipeline unit
TH_N = TL // T_HALF    # 2
CGB_W = 512            # channels per pipeline unit
CGB_N = C // CGB_W     # 4
CG_PER_B = CGB_W // 128  # 4 channel groups of 128 per unit
NCHUNK = (T_HALF + HALO + 127) // 128  # 17

_CACHE = {}


def _build_nc():
    import concourse.bacc as bacc
    import concourse.mybir as mybir
    from concourse.tile import TileContext

    f16 = mybir.dt.float16
    f32 = mybir.dt.float32
    AF = mybir.ActivationFunctionType
    OP = mybir.AluOpType

    nc = bacc.Bacc("TRN2", target_bir_lowering=False, debug=False,
                   num_devices=N_CORES, name="causal_dwconv1d")

    x = nc.dram_tensor("x", [PAD_ROWS, C], f32, kind="ExternalInput")
    w = nc.dram_tensor("w", [128, C // 128, K], f32, kind="ExternalInput")
    bias = nc.dram_tensor("bias", [128, C // 128], f32, kind="ExternalInput")
    ident = nc.dram_tensor("ident", [128, 128], f16, kind="ExternalInput")
    out = nc.dram_tensor("out", [TL, C], f32, kind="ExternalOutput")

    with TileContext(nc) as tc:
        with (
            tc.tile_pool(name="const", bufs=1) as cpool,
            tc.tile_pool(name="stage", bufs=2) as spool,
            tc.tile_pool(name="work", bufs=2) as wpool,
            tc.tile_pool(name="ostage", bufs=2) as opool,
            tc.tile_pool(name="xt_psum", bufs=2, space="PSUM") as xtpool,
            tc.tile_pool(name="o_psum", bufs=2, space="PSUM") as oppool,
        ):
            w_sb = cpool.tile([128, C // 128, K], f32, tag="w")
            nc.sync.dma_start(out=w_sb, in_=w.ap())
            bias_sb = cpool.tile([128, C // 128], f32, tag="bias")
            nc.sync.dma_start(out=bias_sb, in_=bias.ap())
            id_sb = cpool.tile([128, 128], f16, tag="ident")
            nc.sync.dma_start(out=id_sb, in_=ident.ap())

            for th in range(TH_N):
                r0 = th * T_HALF
                for cgb in range(CGB_N):
                    c0 = cgb * CGB_W
                    # ---- load [2176, 512] fp32 -> fp16 staged as [128, 17, 512]
                    stage = spool.tile([128, NCHUNK, CGB_W], f16, tag="stage")
                    src = x[r0:r0 + NCHUNK * 128, c0:c0 + CGB_W]
                    nc.gpsimd.dma_start(
                        out=stage, in_=src.rearrange("(j p) c -> p j c", p=128)
                    )

                    outT_list = []
                    for cg_l in range(CG_PER_B):
                        cg = cgb * CG_PER_B + cg_l
                        # ---- transpose into PSUM: xT[c(128), t(2176)]
                        xt = xtpool.tile([128, NCHUNK * 128], f16, tag="xt")
                        for j in range(NCHUNK):
                            nc.tensor.transpose(
                                xt[:, j * 128:(j + 1) * 128],
                                stage[:, j, cg_l * 128:(cg_l + 1) * 128],
                                id_sb,
                            )
                        # ---- MAC: out[c,i] = sum_k w[k,c]*xT[c,i+k] + bias[c]
                        y13 = wpool.tile([128, T_HALF], f16, tag="y13")
                        nc.scalar.activation(
                            y13, xt[:, 1:1 + T_HALF], AF.Identity,
                            bias=bias_sb[:, cg:cg + 1], scale=w_sb[:, cg, 1:2],
                        )
                        y3 = wpool.tile([128, T_HALF], f16, tag="y3")
                        nc.scalar.activation(
                            y3, xt[:, 3:3 + T_HALF], AF.Identity,
                            bias=0.0, scale=w_sb[:, cg, 3:4],
                        )
                        acc1 = wpool.tile([128, T_HALF], f16, tag="acc1")
                        nc.vector.scalar_tensor_tensor(
                            out=acc1, in0=xt[:, 0:T_HALF], scalar=w_sb[:, cg, 0:1],
                            in1=y13, op0=OP.mult, op1=OP.add,
                        )
                        acc2 = wpool.tile([128, T_HALF], f16, tag="acc2")
                        nc.vector.scalar_tensor_tensor(
                            out=acc2, in0=xt[:, 2:2 + T_HALF], scalar=w_sb[:, cg, 2:3],
                            in1=acc1, op0=OP.mult, op1=OP.add,
                        )
                        outT = wpool.tile([128, T_HALF], f16, tag=f"outT{cg_l}")
                        nc.vector.tensor_add(out=outT, in0=acc2, in1=y3)
                        outT_list.append(outT)

                    # ---- transpose back: for each 128-t block, 4 cg transposes
                    ost = opool.tile([128, T_HALF // 128, CGB_W], f16, tag="ost")
                    for m in range(T_HALF // 128):
                        op = oppool.tile([128, CGB_W], f16, tag="opsum")
                        for cg_l in range(CG_PER_B):
                            nc.tensor.transpose(
                                op[:, cg_l * 128:(cg_l + 1) * 128],
                                outT_list[cg_l][:, m * 128:(m + 1) * 128],
                                id_sb,
                            )
                        nc.vector.tensor_copy(out=ost[:, m, :], in_=op)

                    # ---- store [2048, 512] fp16 -> fp32
                    dst = out[r0:r0 + T_HALF, c0:c0 + CGB_W]
                    nc.gpsimd.dma_start(
                        out=dst.rearrange("(m p) c -> p m c", p=128), in_=ost
                    )

    nc.compile()
    return nc


def _get_nc():
    if "nc" not in _CACHE:
        _CACHE["nc"] = _build_nc()
    return _CACHE["nc"]


def _host_inputs(x, weight, bias):
    x = np.asarray(x, dtype=np.float32)
    weight = np.asarray(weight, dtype=np.float32)
    bias = np.asarray(bias, dtype=np.float32)

    # padded rows per batch: HALO zeros, then T rows of x, then tail zeros
    pad_total = HALO + T + (PAD_ROWS - HALO - TL)  # 3 + 8192 + 125 = 8320
    xp = np.zeros((B, pad_total, C), dtype=np.float32)
    xp[:, HALO:HALO + T, :] = x

    # weights: [K,1,C] -> [128, C//128, K]
    w_t = weight[:, 0, :].T.reshape(C // 128, 128, K).transpose(1, 0, 2)
    w_t = np.ascontiguousarray(w_t, dtype=np.float32)
    b_t = np.ascontiguousarray(
        bias.reshape(C // 128, 128).T, dtype=np.float32
    )
    id16 = np.eye(128, dtype=np.float16)

    in_maps = []
    for core in range(N_CORES):
        b, h = divmod(core, 2)
        shard = np.ascontiguousarray(xp[b, h * TL:h * TL + PAD_ROWS, :])
        in_maps.append({"x": shard, "w": w_t, "bias": b_t, "ident": id16})
    return in_maps


def kernel(x, weight, bias):
    from concourse import bass2jax

    nc = _get_nc()
    in_maps = _host_inputs(x, weight, bias)
    results = bass2jax.run_bass_via_pjrt(nc, in_maps, n_cores=N_CORES)

    out = np.empty((B, T, C), dtype=np.float32)
    for core in range(N_CORES):
        b, h = divmod(core, 2)
        out[b, h * TL:(h + 1) * TL, :] = results[core]["out"]
    return out


# revision 7
# speedup vs baseline: 86.0664x; 86.0664x over previous
"""Causal depthwise conv1d (B=4, T=8192, C=2048, K=4) on 8 Trainium2 cores.

Sharding: 8 shards = (batch b, T-half h), each core computes out[b, h*4096:(h+1)*4096, :].
Halo handled host-side: each core's input is 4224 rows of a zero-padded copy of x,
so row i of the shard is x[b, t0 + i - 3] (zeros outside [0, T)).

Per-core kernel (all fp16 on-chip, fp32 in HBM):
  - SWDGE DMA loads [t,c] chunks with fp32->fp16 cast (contiguous 2KB HBM reads)
  - PE transposes 128x128 chunks into PSUM => xT[c_part, t_free]
  - MAC with per-partition scalars: out[c,t] = sum_k w_k[c]*xT[c,t+k] + bias[c]
      ACT takes odd offsets (alignment-immune), DVE takes even offsets via
      fused scalar_tensor_tensor; all tensor operands 16-bit => 2x/4x DVE modes
  - PE transposes the result back to [t,c], DVE evacuates PSUM
  - SWDGE DMA stores with fp16->fp32 cast
"""

import sys

if "/opt/trn_rl_repo" not in sys.path:
    sys.path.insert(0, "/opt/trn_rl_repo")

import numpy as np

B, T, C, K = 4, 8192, 2048, 4
N_CORES = 8
TL = T // 2            # 4096 rows of output per core
HALO = K - 1           # 3
PAD_ROWS = TL + 128    # 4224 input rows per core (halo + data + tail pad)
T_HALF = 2048          # time rows per pipeline unit
TH_N = TL // T_HALF    # 2
CGB_W = 512            # channels per pipeline unit
CGB_N = C // CGB_W     # 4
CG_PER_B = CGB_W // 128  # 4 channel groups of 128 per unit
NCHUNK = (T_HALF + HALO + 127) // 128  # 17

_CACHE = {}


def _build_nc(reps=1):
    import concourse.bacc as bacc
    import concourse.mybir as mybir
    from concourse.tile import TileContext

    f16 = mybir.dt.float16
    f32 = mybir.dt.float32
    AF = mybir.ActivationFunctionType
    OP = mybir.AluOpType

    nc = bacc.Bacc("TRN2", target_bir_lowering=False, debug=False,
                   num_devices=N_CORES, name="causal_dwconv1d")

    x = nc.dram_tensor("x", [PAD_ROWS, C], f32, kind="ExternalInput")
    w = nc.dram_tensor("w", [128, C // 128, K], f32, kind="ExternalInput")
    bias = nc.dram_tensor("bias", [128, C // 128], f32, kind="ExternalInput")
    ident = nc.dram_tensor("ident", [128, 128], f16, kind="ExternalInput")
    out = nc.dram_tensor("out", [TL, C], f32, kind="ExternalOutput")

    with TileContext(nc) as tc:
        with (
            tc.tile_pool(name="const", bufs=1) as cpool,
            tc.tile_pool(name="stage", bufs=2) as spool,
            tc.tile_pool(name="work", bufs=2) as wpool,
            tc.tile_pool(name="ostage", bufs=2) as opool,
            tc.tile_pool(name="xt_psum", bufs=2, space="PSUM") as xtpool,
            tc.tile_pool(name="o_psum", bufs=2, space="PSUM") as oppool,
        ):
            w_sb = cpool.tile([128, C // 128, K], f32, tag="w")
            nc.sync.dma_start(out=w_sb, in_=w.ap())
            bias_sb = cpool.tile([128, C // 128], f32, tag="bias")
            nc.sync.dma_start(out=bias_sb, in_=bias.ap())
            id_sb = cpool.tile([128, 128], f16, tag="ident")
            nc.sync.dma_start(out=id_sb, in_=ident.ap())

            from contextlib import nullcontext
            loop = tc.For_i(0, reps, 1) if reps > 1 else nullcontext()
            with loop:
              for th in range(TH_N):
                r0 = th * T_HALF
                for cgb in range(CGB_N):
                    c0 = cgb * CGB_W
                    # ---- load [2176, 512] fp32 -> fp16 staged as [128, 17, 512]
                    stage = spool.tile([128, NCHUNK, CGB_W], f16, tag="stage")
                    src = x[r0:r0 + NCHUNK * 128, c0:c0 + CGB_W]
                    nc.gpsimd.dma_start(
                        out=stage, in_=src.rearrange("(j p) c -> p j c", p=128)
                    )

                    outT_list = []
                    for cg_l in range(CG_PER_B):
                        cg = cgb * CG_PER_B + cg_l
                        # ---- transpose into PSUM: xT[c(128), t(2176)]
                        xt = xtpool.tile([128, NCHUNK * 128], f16, tag="xt")
                        for j in range(NCHUNK):
                            nc.tensor.transpose(
                                xt[:, j * 128:(j + 1) * 128],
                                stage[:, j, cg_l * 128:(cg_l + 1) * 128],
                                id_sb,
                            )
                        # ---- MAC: out[c,i] = sum_k w[k,c]*xT[c,i+k] + bias[c]
                        y13 = wpool.tile([128, T_HALF], f16, tag="y13")
                        nc.scalar.activation(
                            y13, xt[:, 1:1 + T_HALF], AF.Identity,
                            bias=bias_sb[:, cg:cg + 1], scale=w_sb[:, cg, 1:2],
                        )
                        y3 = wpool.tile([128, T_HALF], f16, tag="y3")
                        nc.scalar.activation(
                            y3, xt[:, 3:3 + T_HALF], AF.Identity,
                            bias=0.0, scale=w_sb[:, cg, 3:4],
                        )
                        acc1 = wpool.tile([128, T_HALF], f16, tag="acc1")
                        nc.vector.scalar_tensor_tensor(
                            out=acc1, in0=xt[:, 0:T_HALF], scalar=w_sb[:, cg, 0:1],
                            in1=y13, op0=OP.mult, op1=OP.add,
                        )
                        acc2 = wpool.tile([128, T_HALF], f16, tag="acc2")
                        nc.vector.scalar_tensor_tensor(
                            out=acc2, in0=xt[:, 2:2 + T_HALF], scalar=w_sb[:, cg, 2:3],
                            in1=acc1, op0=OP.mult, op1=OP.add,
                        )
                        outT = wpool.tile([128, T_HALF], f16, tag=f"outT{cg_l}")
                        nc.vector.tensor_add(out=outT, in0=acc2, in1=y3)
                        outT_list.append(outT)

                    # ---- transpose back: for each 128-t block, 4 cg transposes
                    ost = opool.tile([128, T_HALF // 128, CGB_W], f16, tag="ost")
                    for m in range(T_HALF // 128):
                        op = oppool.tile([128, CGB_W], f16, tag="opsum")
                        for cg_l in range(CG_PER_B):
                            nc.tensor.transpose(
                                op[:, cg_l * 128:(cg_l + 1) * 128],
                                outT_list[cg_l][:, m * 128:(m + 1) * 128],
                                id_sb,
                            )
                        nc.vector.tensor_copy(out=ost[:, m, :], in_=op)

                    # ---- store [2048, 512] fp16 -> fp32
                    dst = out[r0:r0 + T_HALF, c0:c0 + CGB_W]
                    nc.gpsimd.dma_start(
                        out=dst.rearrange("(m p) c -> p m c", p=128), in_=ost
                    )

    nc.compile()
    return nc


def _get_nc(reps=1):
    if reps not in _CACHE:
        _CACHE[reps] = _build_nc(reps)
    return _CACHE[reps]


def _host_inputs(x, weight, bias):
    x = np.asarray(x, dtype=np.float32)
    weight = np.asarray(weight, dtype=np.float32)
    bias = np.asarray(bias, dtype=np.float32)

    # padded rows per batch: HALO zeros, then T rows of x, then tail zeros
    pad_total = HALO + T + (PAD_ROWS - HALO - TL)  # 3 + 8192 + 125 = 8320
    xp = np.zeros((B, pad_total, C), dtype=np.float32)
    xp[:, HALO:HALO + T, :] = x

    # weights: [K,1,C] -> [128, C//128, K]
    w_t = weight[:, 0, :].T.reshape(C // 128, 128, K).transpose(1, 0, 2)
    w_t = np.ascontiguousarray(w_t, dtype=np.float32)
    b_t = np.ascontiguousarray(
        bias.reshape(C // 128, 128).T, dtype=np.float32
    )
    id16 = np.eye(128, dtype=np.float16)

    in_maps = []
    for core in range(N_CORES):
        b, h = divmod(core, 2)
        shard = np.ascontiguousarray(xp[b, h * TL:h * TL + PAD_ROWS, :])
        in_maps.append({"x": shard, "w": w_t, "bias": b_t, "ident": id16})
    return in_maps


def kernel(x, weight, bias):
    from concourse import bass2jax

    nc = _get_nc()
    in_maps = _host_inputs(x, weight, bias)
    results = bass2jax.run_bass_via_pjrt(nc, in_maps, n_cores=N_CORES)

    out = np.empty((B, T, C), dtype=np.float32)
    for core in range(N_CORES):
        b, h = divmod(core, 2)
        out[b, h * TL:(h + 1) * TL, :] = results[core]["out"]
    return out
